# revision 15
# baseline (speedup 1.0000x reference)
"""ClusterAwareAttention Trainium2 kernel (8 NeuronCores, axon/PJRT path).

Sharding: data-parallel over (batch, sequence-half) -> 8 shards of 8192 rows.

Two launches:
  Pass 1: per-shard cluster pooling partial sums  xp = A_loc^T @ x_loc  (64, 256).
  Host:   reduce halves, build pooled K/V-derived constants:
            Wk_cl (x @ Wk_cl = q @ k_cluster^T * scale, folded through W_q),
            VBDT block-diagonal v_cluster (for attn^T @ V),
            cb_rep2 (cluster bias, 2-head replication).
  Pass 2: per-shard fused attention in transposed (cluster-major) layout:
            logits^T chunks -> exp -> softmax denom via indicator matmul ->
            1/s via ln/exp on ScalarE -> broadcast matmul -> normalize ->
            output projection, all with n on the free axis (512-row groups).

Numerics: x / A / Wk_cl / cluster_bias enter pass 2 as bf16 (the q.k_cl term is
contraction-error dominated either way); everything else fp32/fp32r with fp32
PSUM accumulation.
"""

import json
import os
from functools import lru_cache

import numpy as np

import concourse.bass as bass
import concourse.tile as tile
from concourse import mybir
from concourse.bass_utils import run_bass_kernel_spmd

import ml_dtypes

BF16 = ml_dtypes.bfloat16

B, N, C, H, K = 4, 16384, 256, 8, 64
D = C // H
EPS = 1e-8
SCALE = D ** -0.5
NLOC = N // 2           # rows per shard
F = 512                 # group size (n on the free axis)
NGROUPS = NLOC // F
NCORES = 8

f32 = mybir.dt.float32
f32r = mybir.dt.float32r
bf16 = mybir.dt.bfloat16


# --------------------------------------------------------------------------
# BIR fixup: this container's walrus rejects instructions with >1 sync wait.
# Split extra waits onto single-wait EventSemaphore instructions just before.
# --------------------------------------------------------------------------
def _split_block(bb, counter):
    insts = bb.get("instructions")
    if insts:
        new_insts = []
        for inst in insts:
            si = inst.get("sync_info") or {}
            waits = si.get("on_wait") or []
            if len(waits) > 1:
                for w in waits[:-1]:
                    counter[0] += 1
                    new_insts.append(
                        {
                            "debug": inst.get("debug", 0),
                            "engine": inst["engine"],
                            "ins": [],
                            "name": f"WSPLIT-{counter[0]}",
                            "opcode": "EventSemaphore",
                            "outs": [],
                            "sync_info": {"on_update": [], "on_wait": [w]},
                        }
                    )
                si = dict(si)
                si["on_wait"] = [waits[-1]]
                inst = dict(inst)
                inst["sync_info"] = si
            new_insts.append(inst)
        bb["instructions"] = new_insts
    for sub in bb.get("blocks", []) or []:
        _split_block(sub, counter)


def _fixup_bir_json(bir_json: bytes) -> bytes:
    bir = json.loads(bir_json)
    counter = [0]
    for fn in bir.get("functions", []):
        for bb in fn.get("blocks", []) or []:
            _split_block(bb, counter)
    return json.dumps(bir).encode()


LAST_EXEC_NS = None
TRACE_DIRS = []


def _install_profhook():
    import sys
    import types

    if "antenv.axon_hooks" in sys.modules:
        return
    import antenv

    mod = types.ModuleType("antenv.axon_hooks")
    _hook = [None]
    mod.set_axon_ntff_profile_hook = lambda h: _hook.__setitem__(0, h)
    mod.get_axon_ntff_profile_hook = lambda: _hook[0]
    sys.modules["antenv.axon_hooks"] = mod
    antenv.axon_hooks = mod
    from trn_agent_boot.trn_boot import _ntff_profile_via_ctypes

    mod.set_axon_ntff_profile_hook(
        _ntff_profile_via_ctypes("/opt/axon/libaxon_pjrt.so")
    )


_fixup_installed = False


def _install_fixup():
    global _fixup_installed
    if _fixup_installed:
        return
    _fixup_installed = True
    import concourse.bass_utils as bu
    import concourse.bass2jax as b2j

    orig = bu.compile_bir_kernel

    def patched(bir_json, tmpdir, neff_name="file.neff"):
        return orig(_fixup_bir_json(bir_json), tmpdir, neff_name=neff_name)

    bu.compile_bir_kernel = patched
    b2j.compile_bir_kernel = patched


# --------------------------------------------------------------------------
# Pass 1: xp_part[kcl, c] = sum_n A_loc[n, kcl] * x_loc[n, c]
# --------------------------------------------------------------------------
@lru_cache(maxsize=1)
def _build_pass1():
    nc = bass.Bass()
    x_ext = nc.declare_dram_parameter("x", [NLOC, C], f32r, isOutput=False)
    a_ext = nc.declare_dram_parameter("a", [NLOC, K], f32r, isOutput=False)
    xp_ext = nc.declare_dram_parameter("xp", [K, C], f32, isOutput=True)

    GT = 4                    # tiles of 128 rows per DMA group
    NG = NLOC // (128 * GT)   # 16 groups
    x_r = x_ext.rearrange("(g i p) c -> g p i c", p=128, i=GT)
    a_r = a_ext.rearrange("(g i p) k -> g p i k", p=128, i=GT)

    with tile.TileContext(nc) as tc:
        with (
            tc.tile_pool(name="xin", bufs=3) as xin,
            tc.tile_pool(name="ain", bufs=3) as ain,
            tc.tile_pool(name="acc", bufs=1, space="PSUM") as accp,
            tc.tile_pool(name="outp", bufs=1) as outp,
        ):
            acc = accp.tile([K, C], f32)
            for g in range(NG):
                xg = xin.tile([128, GT, C], f32r)
                ag = ain.tile([128, GT, K], f32r)
                nc.sync.dma_start(out=xg[:], in_=x_r[g])
                nc.sync.dma_start(out=ag[:], in_=a_r[g])
                for i in range(GT):
                    t = g * GT + i
                    nc.tensor.matmul(
                        acc[:],
                        ag[:, i, :],
                        xg[:, i, :],
                        start=(t == 0),
                        stop=(t == NLOC // 128 - 1),
                    )
            xps = outp.tile([K, C], f32)
            nc.vector.tensor_copy(xps[:], acc[:])
            nc.sync.dma_start(out=xp_ext[:], in_=xps[:])
    return nc


# --------------------------------------------------------------------------
# Pass 2: full attention for one shard.
# --------------------------------------------------------------------------
@lru_cache(maxsize=1)
def _build_pass2():
    nc = bass.Bass()
    xs_ext = nc.declare_dram_parameter("xs", [C, NLOC], bf16, isOutput=False)
    as_ext = nc.declare_dram_parameter("as_", [K, NLOC], bf16, isOutput=False)
    wkcl_ext = nc.declare_dram_parameter("wkcl", [C, H * K], bf16, isOutput=False)
    cb2_ext = nc.declare_dram_parameter("cb2", [K, 128], bf16, isOutput=False)
    vbdt_ext = nc.declare_dram_parameter("vbdt", [4, 128, 128], f32r, isOutput=False)
    ind2_ext = nc.declare_dram_parameter("ind2", [128, 4, 128], f32r, isOutput=False)
    i2t64_ext = nc.declare_dram_parameter("i2t64", [128, 128], f32r, isOutput=False)
    wproj_ext = nc.declare_dram_parameter("wproj", [C, C], f32r, isOutput=False)
    bproj_ext = nc.declare_dram_parameter("bproj", [1, C], f32, isOutput=False)
    y_ext = nc.declare_dram_parameter("y", [NLOC, C], f32, isOutput=True)

    wkcl_r = wkcl_ext.rearrange("(ch p) m -> p ch m", p=128)   # (128, 2, 512)
    wproj_r = wproj_ext.rearrange("(ch p) c -> p ch c", p=128)  # (128, 2, 256)
    vbdt_r = vbdt_ext.rearrange("j p m -> p j m")               # (128, 4, 64)
    y_r = y_ext.rearrange("(g t p) c -> g p t c", p=128, t=F // 128)

    _bp = bproj_ext[:]
    bproj_bcast = bass.AP(
        tensor=_bp.tensor,
        offset=_bp.offset,
        ap=[[0, 128], _bp.ap[1]],
    )  # (128, C) partition-broadcast read of b_proj

    with tile.TileContext(nc) as tc:
        with (
            tc.tile_pool(name="const", bufs=1) as const,
            tc.tile_pool(name="xt", bufs=2) as xtp,
            tc.tile_pool(name="at", bufs=2) as atp,
            tc.tile_pool(name="big", bufs=2, space="PSUM") as bigp,
            tc.tile_pool(name="pp", bufs=2) as pp,
            tc.tile_pool(name="spp", bufs=2, space="PSUM") as spp,
            tc.tile_pool(name="lnp", bufs=2) as lnp,
            tc.tile_pool(name="rp", bufs=2) as rp,
            tc.tile_pool(name="bsb", bufs=2) as bsb,
            tc.tile_pool(name="xon", bufs=2) as xon,
            tc.tile_pool(name="yp", bufs=1, space="PSUM") as ypp,
            tc.tile_pool(name="ysb", bufs=2) as ysb,
        ):
            wkcl = const.tile([128, 2, H * K], bf16)
            nc.sync.dma_start(out=wkcl[:], in_=wkcl_r[:])
            cb2 = const.tile([K, 128], bf16)
            nc.sync.dma_start(out=cb2[:], in_=cb2_ext[:])
            vbdt = const.tile([128, 4, 128], f32r)
            nc.sync.dma_start(out=vbdt[:], in_=vbdt_r[:])
            ind2 = const.tile([128, 4, 128], f32r)
            nc.sync.dma_start(out=ind2[:], in_=ind2_ext[:])
            i2t64 = const.tile([128, 128], f32r)
            nc.sync.dma_start(out=i2t64[:], in_=i2t64_ext[:])
            wproj = const.tile([128, 2, C], f32r)
            nc.sync.dma_start(out=wproj[:], in_=wproj_r[:])
            bpt = const.tile([128, C], f32)
            nc.sync.dma_start(out=bpt[:], in_=bproj_bcast)

            for g in range(NGROUPS):
                n0 = g * F
                # ---- transposed loads (host pre-transposed, bf16) ----
                xT = xtp.tile([128, 2, F], bf16)
                for ch in range(2):
                    nc.sync.dma_start(
                        out=xT[:, ch, :],
                        in_=xs_ext[ch * 128 : (ch + 1) * 128, n0 : n0 + F],
                    )
                aT = atp.tile([K, F], bf16)
                nc.sync.dma_start(out=aT[:], in_=as_ext[:, n0 : n0 + F])
                # ---- logits^T chunks: (128 (h2,kcl), F) x 4, pairs per psum tile
                lg = bigp.tile([128, 2, F], f32, tag="big")
                lg2 = bigp.tile([128, 2, F], f32, tag="big")
                for m in range(4):
                    dst = lg[:, m, :] if m < 2 else lg2[:, m - 2, :]
                    nc.tensor.matmul(
                        dst, wkcl[:, 0, 128 * m : 128 * (m + 1)], xT[:, 0, :],
                        start=True, stop=False,
                    )
                    nc.tensor.matmul(
                        dst, wkcl[:, 1, 128 * m : 128 * (m + 1)], xT[:, 1, :],
                        start=False, stop=False,
                    )
                    nc.tensor.matmul(
                        dst, cb2[:], aT[:], start=False, stop=True,
                    )
                # ---- probabilities (unnormalized): exp on ScalarE ----
                P = pp.tile([128, 2, F], f32r)
                P2 = pp.tile([128, 2, F], f32r, tag="P")
                nc.scalar.activation(
                    P[:].rearrange("p a b -> p (a b)"),
                    lg[:].rearrange("p a b -> p (a b)"),
                    mybir.ActivationFunctionType.Exp,
                )
                nc.scalar.activation(
                    P2[:].rearrange("p a b -> p (a b)"),
                    lg2[:].rearrange("p a b -> p (a b)"),
                    mybir.ActivationFunctionType.Exp,
                )
                # ---- softmax denominators, padded rows 32j+{0,1} ----
                spad = spp.tile([128, F], f32)
                for j in range(4):
                    Pj = P[:, j, :] if j < 2 else P2[:, j - 2, :]
                    nc.tensor.matmul(
                        spad[:], ind2[:, j, :], Pj,
                        start=(j == 0), stop=(j == 3),
                    )
                # ---- r = exp(-ln(s)) on ScalarE (reciprocal table is banned)
                lns = lnp.tile([128, F], f32)
                nc.scalar.activation(
                    lns[:], spad[:], mybir.ActivationFunctionType.Ln
                )
                r = rp.tile([128, F], f32r)
                nc.scalar.activation(
                    r[:], lns[:], mybir.ActivationFunctionType.Exp, scale=-1.0
                )
                # ---- B broadcast + xout^T (both PE), normalize on DVE ----
                Bt = bigp.tile([128, 2, F], f32, tag="big")
                Xt = bigp.tile([128, 2, F], f32, tag="big")
                for t in range(2):
                    for jj in range(2):
                        j = 2 * t + jj
                        Pj = P[:, j, :] if j < 2 else P2[:, j - 2, :]
                        nc.tensor.matmul(
                            Bt[:, t, :],
                            i2t64[32 * j : 32 * j + 2, :],
                            r[32 * j : 32 * j + 2, :],
                            start=(jj == 0), stop=(jj == 1),
                            tile_position=(32 * j, 0),
                        )
                        nc.tensor.matmul(
                            Xt[:, t, :],
                            vbdt[:, j, :], Pj,
                            start=(jj == 0), stop=(jj == 1),
                        )
                Bsb = bsb.tile([128, 2, F], f32)
                nc.vector.tensor_copy(
                    Bsb[:].rearrange("p a b -> p (a b)"),
                    Bt[:].rearrange("p a b -> p (a b)"),
                )
                xoutTn = xon.tile([128, 2, F], f32r)
                nc.vector.tensor_mul(
                    xoutTn[:].rearrange("p a b -> p (a b)"),
                    Xt[:].rearrange("p a b -> p (a b)"),
                    Bsb[:].rearrange("p a b -> p (a b)"),
                )
                # ---- output projection + bias ----
                yt = ypp.tile([128, F // 128, C], f32)
                for t in range(F // 128):
                    for ch in range(2):
                        nc.tensor.matmul(
                            yt[:, t, :],
                            xoutTn[:, ch, t * 128 : (t + 1) * 128],
                            wproj[:, ch, :],
                            start=(ch == 0), stop=(ch == 1),
                        )
                ys = ysb.tile([128, F // 128, C], f32)
                _bpap = bpt[:]
                bpt_b = bass.AP(
                    tensor=_bpap.tensor,
                    offset=_bpap.offset,
                    ap=[_bpap.ap[0], [0, F // 128], _bpap.ap[1]],
                )
                nc.vector.tensor_add(ys[:], yt[:], bpt_b)
                nc.sync.dma_start(out=y_r[g], in_=ys[:])
    return nc


# --------------------------------------------------------------------------
# Host orchestration
# --------------------------------------------------------------------------
def kernel(
    voxel_features,
    cluster_assignments,
    w_qkv,
    w_proj,
    b_proj,
    cluster_bias,
):
    _install_fixup()
    x_all = np.ascontiguousarray(np.asarray(voxel_features, dtype=np.float32))
    A_all = np.ascontiguousarray(np.asarray(cluster_assignments, dtype=np.float32))
    w_qkv = np.asarray(w_qkv, dtype=np.float32)
    w_proj_np = np.ascontiguousarray(np.asarray(w_proj, dtype=np.float32))
    b_proj_np = np.asarray(b_proj, dtype=np.float32)
    cb = np.asarray(cluster_bias, dtype=np.float32)

    W_q = w_qkv[:, 0:C]
    W_k = w_qkv[:, C : 2 * C]
    W_v = w_qkv[:, 2 * C : 3 * C]

    # ---------------- pass 1 ----------------
    nc1 = _build_pass1()
    in_maps1 = []
    for core in range(NCORES):
        b, half = core // 2, core % 2
        in_maps1.append(
            {
                "x": np.ascontiguousarray(x_all[b, half * NLOC : (half + 1) * NLOC]),
                "a": np.ascontiguousarray(A_all[b, half * NLOC : (half + 1) * NLOC]),
            }
        )
    trace = bool(os.environ.get("BASS_PROFILE"))
    if trace:
        _install_profhook()
    global LAST_EXEC_NS, TRACE_DIRS
    TRACE_DIRS = []
    kw1 = {}
    if trace:
        import tempfile
        d = tempfile.mkdtemp(prefix="p1_trace_")
        TRACE_DIRS.append(d)
        kw1 = dict(trace=True, tmpdir=d)
    res1 = run_bass_kernel_spmd(nc1, in_maps1, list(range(NCORES)), **kw1)
    exec1 = getattr(res1, "exec_time_ns", None)
    xp_parts = np.stack([res1.results[c]["xp"] for c in range(NCORES)])  # (8,64,256)

    # ---------------- host glue ----------------
    denom = A_all.sum(axis=1) + EPS  # (B, K)

    IND2 = np.zeros((128, 4, 128), np.float32)
    _eps_cols = np.ones(128, bool)
    for _j in range(4):
        _eps_cols[32 * _j] = False
        _eps_cols[32 * _j + 1] = False
    for _j in range(4):
        IND2[0:64, _j, 32 * _j] = 1.0
        IND2[64:128, _j, 32 * _j + 1] = 1.0
        IND2[:, _j, _eps_cols] = 0.5 / 256.0
    I2T64 = np.zeros((128, 128), np.float32)
    for _j in range(4):
        for _h2 in range(2):
            _c0 = (_j % 2) * 64 + _h2 * 32
            I2T64[32 * _j + _h2, _c0 : _c0 + 32] = 1.0
    cb2 = np.zeros((K, 128), np.float32)
    cb2[:, 0:64] = cb
    cb2[:, 64:128] = cb

    Wk_cl_all = []
    VBDT_all = []
    Wq3 = W_q.reshape(C, H, D)
    for b in range(B):
        xp = xp_parts[2 * b] + xp_parts[2 * b + 1]
        pooled = xp / denom[b][:, None]
        k_cl = pooled @ W_k
        v_cl = pooled @ W_v
        k3 = k_cl.reshape(K, H, D)
        Wk_cl = np.einsum("chd,khd->chk", Wq3, k3).reshape(C, H * K) * SCALE
        Wk_cl_all.append(Wk_cl.astype(BF16))
        v3 = v_cl.reshape(K, H, D)
        VBDT = np.zeros((4, 128, 128), np.float32)
        for j in range(4):
            for h2 in range(2):
                c0 = (j % 2) * 64 + h2 * 32
                VBDT[j, h2 * 64 : (h2 + 1) * 64, c0 : c0 + 32] = v3[
                    :, 2 * j + h2, :
                ]
        VBDT_all.append(VBDT)

    # ---------------- pass 2 ----------------
    nc2 = _build_pass2()
    in_maps2 = []
    cb2_bf = cb2.astype(BF16)
    for core in range(NCORES):
        b, half = core // 2, core % 2
        in_maps2.append(
            {
                "xs": np.ascontiguousarray(
                    x_all[b, half * NLOC : (half + 1) * NLOC].astype(BF16).T
                ),
                "as_": np.ascontiguousarray(
                    A_all[b, half * NLOC : (half + 1) * NLOC].astype(BF16).T
                ),
                "wkcl": Wk_cl_all[b],
                "cb2": cb2_bf,
                "vbdt": VBDT_all[b],
                "ind2": IND2,
                "i2t64": I2T64,
                "wproj": w_proj_np,
                "bproj": b_proj_np.reshape(1, C),
            }
        )
    kw2 = {}
    if trace:
        import tempfile
        d = tempfile.mkdtemp(prefix="p2_trace_")
        TRACE_DIRS.append(d)
        kw2 = dict(trace=True, tmpdir=d)
    res2 = run_bass_kernel_spmd(nc2, in_maps2, list(range(NCORES)), **kw2)
    exec2 = getattr(res2, "exec_time_ns", None)
    if exec1 is not None and exec2 is not None:
        LAST_EXEC_NS = exec1 + exec2
        LAST_EXEC_SPLIT = (exec1, exec2)
        globals()["LAST_EXEC_SPLIT"] = LAST_EXEC_SPLIT

    y_out = np.zeros((B, N, C), np.float32)
    for core in range(NCORES):
        b, half = core // 2, core % 2
        y_out[b, half * NLOC : (half + 1) * NLOC] = res2.results[core]["y"]
    return y_out


# revision 18
# speedup vs baseline: 1.2442x; 1.2442x over previous
"""ClusterAwareAttention Trainium2 kernel (8 NeuronCores, axon/PJRT path).

Sharding: data-parallel over (batch, sequence-half) -> 8 shards of 8192 rows.

Two launches:
  Pass 1: per-shard cluster pooling partial sums  xp = A_loc^T @ x_loc  (64, 256).
  Host:   reduce halves, build pooled K/V-derived constants:
            Wk_cl (x @ Wk_cl = q @ k_cluster^T * scale, folded through W_q),
            VBDT block-diagonal v_cluster (for attn^T @ V),
            cb_rep2 (cluster bias, 2-head replication).
  Pass 2: per-shard fused attention in transposed (cluster-major) layout:
            logits^T chunks -> exp -> softmax denom via indicator matmul ->
            1/s via ln/exp on ScalarE -> broadcast matmul -> normalize ->
            output projection, all with n on the free axis (512-row groups).

Numerics: x / A / Wk_cl / cluster_bias enter pass 2 as bf16 (the q.k_cl term is
contraction-error dominated either way); everything else fp32/fp32r with fp32
PSUM accumulation.
"""

import json
import os
from functools import lru_cache

import numpy as np

import concourse.bass as bass
import concourse.tile as tile
from concourse import mybir
from concourse.bass_utils import run_bass_kernel_spmd

import ml_dtypes

BF16 = ml_dtypes.bfloat16

B, N, C, H, K = 4, 16384, 256, 8, 64
D = C // H
EPS = 1e-8
SCALE = D ** -0.5
NLOC = N // 2           # rows per shard
F = 512                 # group size (n on the free axis)
NGROUPS = NLOC // F
NCORES = 8

f32 = mybir.dt.float32
f32r = mybir.dt.float32r
bf16 = mybir.dt.bfloat16


# --------------------------------------------------------------------------
# BIR fixup: this container's walrus rejects instructions with >1 sync wait.
# Split extra waits onto single-wait EventSemaphore instructions just before.
# --------------------------------------------------------------------------
def _split_block(bb, counter):
    insts = bb.get("instructions")
    if insts:
        new_insts = []
        for inst in insts:
            si = inst.get("sync_info") or {}
            waits = si.get("on_wait") or []
            if len(waits) > 1:
                for w in waits[:-1]:
                    counter[0] += 1
                    new_insts.append(
                        {
                            "debug": inst.get("debug", 0),
                            "engine": inst["engine"],
                            "ins": [],
                            "name": f"WSPLIT-{counter[0]}",
                            "opcode": "EventSemaphore",
                            "outs": [],
                            "sync_info": {"on_update": [], "on_wait": [w]},
                        }
                    )
                si = dict(si)
                si["on_wait"] = [waits[-1]]
                inst = dict(inst)
                inst["sync_info"] = si
            new_insts.append(inst)
        bb["instructions"] = new_insts
    for sub in bb.get("blocks", []) or []:
        _split_block(sub, counter)


def _fixup_bir_json(bir_json: bytes) -> bytes:
    bir = json.loads(bir_json)
    counter = [0]
    for fn in bir.get("functions", []):
        for bb in fn.get("blocks", []) or []:
            _split_block(bb, counter)
    return json.dumps(bir).encode()


LAST_EXEC_NS = None
TRACE_DIRS = []


def _install_profhook():
    import sys
    import types

    if "antenv.axon_hooks" in sys.modules:
        return
    import antenv

    mod = types.ModuleType("antenv.axon_hooks")
    _hook = [None]
    mod.set_axon_ntff_profile_hook = lambda h: _hook.__setitem__(0, h)
    mod.get_axon_ntff_profile_hook = lambda: _hook[0]
    sys.modules["antenv.axon_hooks"] = mod
    antenv.axon_hooks = mod
    from trn_agent_boot.trn_boot import _ntff_profile_via_ctypes

    mod.set_axon_ntff_profile_hook(
        _ntff_profile_via_ctypes("/opt/axon/libaxon_pjrt.so")
    )


_fixup_installed = False


def _install_fixup():
    global _fixup_installed
    if _fixup_installed:
        return
    _fixup_installed = True
    import concourse.bass_utils as bu
    import concourse.bass2jax as b2j

    orig = bu.compile_bir_kernel

    def patched(bir_json, tmpdir, neff_name="file.neff"):
        return orig(_fixup_bir_json(bir_json), tmpdir, neff_name=neff_name)

    bu.compile_bir_kernel = patched
    b2j.compile_bir_kernel = patched


# --------------------------------------------------------------------------
# Pass 1: xp_part[kcl, c] = sum_n A_loc[n, kcl] * x_loc[n, c]
# --------------------------------------------------------------------------
@lru_cache(maxsize=1)
def _build_pass1():
    nc = bass.Bass()
    x_ext = nc.declare_dram_parameter("x", [NLOC, C], f32r, isOutput=False)
    a_ext = nc.declare_dram_parameter("a", [NLOC, K], f32r, isOutput=False)
    xp_ext = nc.declare_dram_parameter("xp", [K, C], f32, isOutput=True)

    GT = 4                    # tiles of 128 rows per DMA group
    NG = NLOC // (128 * GT)   # 16 groups
    x_r = x_ext.rearrange("(g i p) c -> g p i c", p=128, i=GT)
    a_r = a_ext.rearrange("(g i p) k -> g p i k", p=128, i=GT)

    with tile.TileContext(nc) as tc:
        with (
            tc.tile_pool(name="xin", bufs=3) as xin,
            tc.tile_pool(name="ain", bufs=3) as ain,
            tc.tile_pool(name="acc", bufs=1, space="PSUM") as accp,
            tc.tile_pool(name="outp", bufs=1) as outp,
        ):
            acc = accp.tile([K, C], f32)
            for g in range(NG):
                xg = xin.tile([128, GT, C], f32r)
                ag = ain.tile([128, GT, K], f32r)
                nc.sync.dma_start(out=xg[:], in_=x_r[g])
                nc.sync.dma_start(out=ag[:], in_=a_r[g])
                for i in range(GT):
                    t = g * GT + i
                    nc.tensor.matmul(
                        acc[:],
                        ag[:, i, :],
                        xg[:, i, :],
                        start=(t == 0),
                        stop=(t == NLOC // 128 - 1),
                    )
            xps = outp.tile([K, C], f32)
            nc.vector.tensor_copy(xps[:], acc[:])
            nc.sync.dma_start(out=xp_ext[:], in_=xps[:])
    return nc


# --------------------------------------------------------------------------
# Pass 2: full attention for one shard.
# --------------------------------------------------------------------------
@lru_cache(maxsize=1)
def _build_pass2():
    nc = bass.Bass()
    xs_ext = nc.declare_dram_parameter("xs", [C, NLOC], bf16, isOutput=False)
    as_ext = nc.declare_dram_parameter("as_", [K, NLOC], bf16, isOutput=False)
    wkcl_ext = nc.declare_dram_parameter("wkcl", [C, H * K], bf16, isOutput=False)
    cb2_ext = nc.declare_dram_parameter("cb2", [K, 128], bf16, isOutput=False)
    vbdt_ext = nc.declare_dram_parameter("vbdt", [4, 128, 128], f32r, isOutput=False)
    ind2_ext = nc.declare_dram_parameter("ind2", [128, 4, 128], f32r, isOutput=False)
    wproj_ext = nc.declare_dram_parameter("wproj", [C, C], f32r, isOutput=False)
    bproj_ext = nc.declare_dram_parameter("bproj", [1, C], f32, isOutput=False)
    y_ext = nc.declare_dram_parameter("y", [NLOC, C], f32, isOutput=True)

    wkcl_r = wkcl_ext.rearrange("(ch p) m -> p ch m", p=128)   # (128, 2, 512)
    wproj_r = wproj_ext.rearrange("(ch p) c -> p ch c", p=128)  # (128, 2, 256)
    vbdt_r = vbdt_ext.rearrange("j p m -> p j m")               # (128, 4, 128)

    _bp = bproj_ext[:]
    bproj_bcast = bass.AP(
        tensor=_bp.tensor,
        offset=_bp.offset,
        ap=[[0, 128], _bp.ap[1]],
    )  # (128, C) partition-broadcast read of b_proj

    with tile.TileContext(nc) as tc:
        with (
            tc.tile_pool(name="const", bufs=1) as const,
            tc.tile_pool(name="xt", bufs=3) as xtp,
            tc.tile_pool(name="at", bufs=3) as atp,
            tc.tile_pool(name="big", bufs=3, space="PSUM") as bigp,
            tc.tile_pool(name="pp", bufs=4) as pp,
            tc.tile_pool(name="spp", bufs=1, space="PSUM") as spp,
            tc.tile_pool(name="rp", bufs=2) as rp,
            tc.tile_pool(name="rrd", bufs=2, space="DRAM") as rrd,
            tc.tile_pool(name="bsb", bufs=2) as bsb,
            tc.tile_pool(name="xon", bufs=2) as xon,
            tc.tile_pool(name="yp", bufs=1, space="PSUM") as ypp,
            tc.tile_pool(name="ysb", bufs=3) as ysb,
        ):
            wkcl = const.tile([128, 2, H * K], bf16)
            nc.sync.dma_start(out=wkcl[:], in_=wkcl_r[:])
            cb2 = const.tile([K, 128], bf16)
            nc.sync.dma_start(out=cb2[:], in_=cb2_ext[:])
            vbdt = const.tile([128, 4, 128], f32r)
            nc.sync.dma_start(out=vbdt[:], in_=vbdt_r[:])
            ind2 = const.tile([128, 4, 128], f32r)
            nc.sync.dma_start(out=ind2[:], in_=ind2_ext[:])
            wproj = const.tile([128, 2, C], f32r)
            nc.sync.dma_start(out=wproj[:], in_=wproj_r[:])
            bpt = const.tile([128, C], f32)
            nc.sync.dma_start(out=bpt[:], in_=bproj_bcast)

            for g in range(NGROUPS):
                n0 = g * F
                # ---- transposed loads (host pre-transposed, bf16) ----
                xT = xtp.tile([128, 2, F], bf16)
                for ch in range(2):
                    nc.sync.dma_start(
                        out=xT[:, ch, :],
                        in_=xs_ext[ch * 128 : (ch + 1) * 128, n0 : n0 + F],
                    )
                aT = atp.tile([K, F], bf16)
                nc.sync.dma_start(out=aT[:], in_=as_ext[:, n0 : n0 + F])
                # ---- logits^T chunks (+ cluster bias), head pairs ----
                lg = bigp.tile([128, 2, F], f32, tag="big")
                lg2 = bigp.tile([128, 2, F], f32, tag="big")
                for m in range(4):
                    dst = lg[:, m, :] if m < 2 else lg2[:, m - 2, :]
                    nc.tensor.matmul(
                        dst, wkcl[:, 0, 128 * m : 128 * (m + 1)], xT[:, 0, :],
                        start=True, stop=False,
                    )
                    nc.tensor.matmul(
                        dst, wkcl[:, 1, 128 * m : 128 * (m + 1)], xT[:, 1, :],
                        start=False, stop=False,
                    )
                    nc.tensor.matmul(
                        dst, cb2[:], aT[:], start=False, stop=True,
                    )
                # ---- unnormalized probabilities ----
                P = pp.tile([128, 2, F], f32r)
                P2 = pp.tile([128, 2, F], f32r, tag="P")
                nc.scalar.activation(
                    P[:].rearrange("p a b -> p (a b)"),
                    lg[:].rearrange("p a b -> p (a b)"),
                    mybir.ActivationFunctionType.Exp,
                )
                nc.scalar.activation(
                    P2[:].rearrange("p a b -> p (a b)"),
                    lg2[:].rearrange("p a b -> p (a b)"),
                    mybir.ActivationFunctionType.Exp,
                )
                # ---- softmax denominators (rows 32j+h2 of spad) ----
                spad = spp.tile([128, F], f32)
                for j in range(4):
                    Pj = P[:, j, :] if j < 2 else P2[:, j - 2, :]
                    nc.tensor.matmul(
                        spad[:], ind2[:, j, :], Pj,
                        start=(j == 0), stop=(j == 3),
                    )
                # ---- r = exp(-ln(s)) on ScalarE ----
                lns = rp.tile([128, F], f32, tag="lns")
                nc.scalar.activation(
                    lns[:], spad[:], mybir.ActivationFunctionType.Ln
                )
                r = rp.tile([128, F], f32, tag="r")
                nc.scalar.activation(
                    r[:], lns[:], mybir.ActivationFunctionType.Exp, scale=-1.0
                )
                # ---- bounce r rows to DRAM, broadcast-load B ----
                rr = rrd.tile([8, F], f32)
                for h2 in range(2):
                    src = bass.AP(
                        tensor=r[:].tensor,
                        offset=r[32 * 0 + h2 : 32 * 0 + h2 + 1, :].offset,
                        ap=[[32 * r[:].ap[0][0], 4]] + [r[:].ap[1]],
                    )
                    dst = rr[:].rearrange("h f -> h f")
                    dsth = bass.AP(
                        tensor=dst.tensor,
                        offset=dst.offset + h2 * F,
                        ap=[[2 * F, 4], [1, F]],
                    )
                    nc.sync.dma_start(out=dsth, in_=src)
                Bs = bsb.tile([128, 2, F], f32)
                for t in range(2):
                    for hh in range(4):
                        h = 4 * t + hh
                        srow = rr[h : h + 1, :]
                        bcast = bass.AP(
                            tensor=srow.tensor,
                            offset=srow.offset,
                            ap=[[0, 32], srow.ap[1]],
                        )
                        nc.sync.dma_start(
                            out=Bs[32 * hh : 32 * (hh + 1), t, :], in_=bcast
                        )
                # ---- xout^T (unnormalized) on PE, then normalize on DVE ----
                Xt = bigp.tile([128, 2, F], f32, tag="big")
                for t in range(2):
                    for jj in range(2):
                        j = 2 * t + jj
                        Pj = P[:, j, :] if j < 2 else P2[:, j - 2, :]
                        nc.tensor.matmul(
                            Xt[:, t, :],
                            vbdt[:, j, :], Pj,
                            start=(jj == 0), stop=(jj == 1),
                        )
                xoutTn = xon.tile([128, 2, F], f32r)
                nc.vector.tensor_mul(
                    xoutTn[:].rearrange("p a b -> p (a b)"),
                    Xt[:].rearrange("p a b -> p (a b)"),
                    Bs[:].rearrange("p a b -> p (a b)"),
                )
                # ---- output projection + bias (two 256-row halves) ----
                for th in range(2):
                    yt = ypp.tile([128, 2, C], f32, tag="y")
                    for t2 in range(2):
                        t = 2 * th + t2
                        for ch in range(2):
                            nc.tensor.matmul(
                                yt[:, t2, :],
                                xoutTn[:, ch, t * 128 : (t + 1) * 128],
                                wproj[:, ch, :],
                                start=(ch == 0), stop=(ch == 1),
                            )
                    ys = ysb.tile([128, 2, C], f32)
                    _bpap = bpt[:]
                    bpt_b = bass.AP(
                        tensor=_bpap.tensor,
                        offset=_bpap.offset,
                        ap=[_bpap.ap[0], [0, 2], _bpap.ap[1]],
                    )
                    nc.vector.tensor_add(ys[:], yt[:], bpt_b)
                    y_dst = y_ext[n0 + th * 256 : n0 + (th + 1) * 256, :]
                    nc.sync.dma_start(
                        out=y_dst.rearrange("(t p) c -> p t c", p=128),
                        in_=ys[:],
                    )
    return nc


# --------------------------------------------------------------------------
# Host orchestration
# --------------------------------------------------------------------------
def kernel(
    voxel_features,
    cluster_assignments,
    w_qkv,
    w_proj,
    b_proj,
    cluster_bias,
):
    _install_fixup()
    x_all = np.ascontiguousarray(np.asarray(voxel_features, dtype=np.float32))
    A_all = np.ascontiguousarray(np.asarray(cluster_assignments, dtype=np.float32))
    w_qkv = np.asarray(w_qkv, dtype=np.float32)
    w_proj_np = np.ascontiguousarray(np.asarray(w_proj, dtype=np.float32))
    b_proj_np = np.asarray(b_proj, dtype=np.float32)
    cb = np.asarray(cluster_bias, dtype=np.float32)

    W_q = w_qkv[:, 0:C]
    W_k = w_qkv[:, C : 2 * C]
    W_v = w_qkv[:, 2 * C : 3 * C]

    # ---------------- pass 1 ----------------
    nc1 = _build_pass1()
    in_maps1 = []
    for core in range(NCORES):
        b, half = core // 2, core % 2
        in_maps1.append(
            {
                "x": np.ascontiguousarray(x_all[b, half * NLOC : (half + 1) * NLOC]),
                "a": np.ascontiguousarray(A_all[b, half * NLOC : (half + 1) * NLOC]),
            }
        )
    trace = bool(os.environ.get("BASS_PROFILE"))
    if trace:
        _install_profhook()
    global LAST_EXEC_NS, TRACE_DIRS
    TRACE_DIRS = []
    kw1 = {}
    if trace:
        import tempfile
        d = tempfile.mkdtemp(prefix="p1_trace_")
        TRACE_DIRS.append(d)
        kw1 = dict(trace=True, tmpdir=d)
    res1 = run_bass_kernel_spmd(nc1, in_maps1, list(range(NCORES)), **kw1)
    exec1 = getattr(res1, "exec_time_ns", None)
    xp_parts = np.stack([res1.results[c]["xp"] for c in range(NCORES)])  # (8,64,256)

    # ---------------- host glue ----------------
    denom = A_all.sum(axis=1) + EPS  # (B, K)

    IND2 = np.zeros((128, 4, 128), np.float32)
    _eps_cols = np.ones(128, bool)
    for _j in range(4):
        _eps_cols[32 * _j] = False
        _eps_cols[32 * _j + 1] = False
    for _j in range(4):
        IND2[0:64, _j, 32 * _j] = 1.0
        IND2[64:128, _j, 32 * _j + 1] = 1.0
        IND2[:, _j, _eps_cols] = 0.5 / 256.0
    I2T64 = np.zeros((128, 128), np.float32)
    for _j in range(4):
        for _h2 in range(2):
            _c0 = (_j % 2) * 64 + _h2 * 32
            I2T64[32 * _j + _h2, _c0 : _c0 + 32] = 1.0
    cb2 = np.zeros((K, 128), np.float32)
    cb2[:, 0:64] = cb
    cb2[:, 64:128] = cb

    Wk_cl_all = []
    VBDT_all = []
    Wq3 = W_q.reshape(C, H, D)
    for b in range(B):
        xp = xp_parts[2 * b] + xp_parts[2 * b + 1]
        pooled = xp / denom[b][:, None]
        k_cl = pooled @ W_k
        v_cl = pooled @ W_v
        k3 = k_cl.reshape(K, H, D)
        Wk_cl = np.einsum("chd,khd->chk", Wq3, k3).reshape(C, H * K) * SCALE
        Wk_cl_all.append(Wk_cl.astype(BF16))
        v3 = v_cl.reshape(K, H, D)
        VBDT = np.zeros((4, 128, 128), np.float32)
        for j in range(4):
            for h2 in range(2):
                c0 = (j % 2) * 64 + h2 * 32
                VBDT[j, h2 * 64 : (h2 + 1) * 64, c0 : c0 + 32] = v3[
                    :, 2 * j + h2, :
                ]
        VBDT_all.append(VBDT)

    # ---------------- pass 2 ----------------
    nc2 = _build_pass2()
    in_maps2 = []
    cb2_bf = cb2.astype(BF16)
    for core in range(NCORES):
        b, half = core // 2, core % 2
        in_maps2.append(
            {
                "xs": np.ascontiguousarray(
                    x_all[b, half * NLOC : (half + 1) * NLOC].astype(BF16).T
                ),
                "as_": np.ascontiguousarray(
                    A_all[b, half * NLOC : (half + 1) * NLOC].astype(BF16).T
                ),
                "wkcl": Wk_cl_all[b],
                "cb2": cb2_bf,
                "vbdt": VBDT_all[b],
                "ind2": IND2,
                "i2t64": I2T64,
                "wproj": w_proj_np,
                "bproj": b_proj_np.reshape(1, C),
            }
        )
    kw2 = {}
    if trace:
        import tempfile
        d = tempfile.mkdtemp(prefix="p2_trace_")
        TRACE_DIRS.append(d)
        kw2 = dict(trace=True, tmpdir=d)
    res2 = run_bass_kernel_spmd(nc2, in_maps2, list(range(NCORES)), **kw2)
    exec2 = getattr(res2, "exec_time_ns", None)
    if exec1 is not None and exec2 is not None:
        LAST_EXEC_NS = exec1 + exec2
        LAST_EXEC_SPLIT = (exec1, exec2)
        globals()["LAST_EXEC_SPLIT"] = LAST_EXEC_SPLIT

    y_out = np.zeros((B, N, C), np.float32)
    for core in range(NCORES):
        b, half = core // 2, core % 2
        y_out[b, half * NLOC : (half + 1) * NLOC] = res2.results[core]["y"]
    return y_out
